# revision 6
# baseline (speedup 1.0000x reference)
"""Multi-head attention (RoPE, causal) Trainium2 Bass kernel, 8-way sharded.

Sharding: core c handles batch b = c//2 and head-group hg = c%2 (8 of 16
heads). Each core computes Q/K/V projections for its head slice in
transposed layout (QT/KT: [hd, l] with de-interleaved RoPE pairs), runs
causal flash-style attention per head with scores kept transposed
(S^T[k, q], keys on partitions), and a partial output projection
out^T = Wo_slice @ attn^T. Host sums the two head-group partials per batch,
transposes back, and adds the output bias.

All matmuls run as float32r (full-rate single-pass fp32 on the PE).
"""

from contextlib import ExitStack

import numpy as np

import concourse.bass as bass
import concourse.mybir as mybir
import concourse.tile as tile
from concourse import bacc
from concourse.bass_utils import run_bass_kernel_spmd

F32 = mybir.dt.float32
F32R = mybir.dt.float32r
AF = mybir.ActivationFunctionType

B, L, D = 4, 2048, 1024
H, HD = 16, 64          # global heads, head dim
HPC = 8                 # heads per core
DH = HPC * HD           # 512: per-core projected width
KT = L // 128           # 16 key tiles
NCORES = 8
ROPE_BASE = 10000.0

_cache: dict = {}


def _r(ap):
    return ap.bitcast(F32R)


def _build(compile=True):
    if "nc" in _cache:
        return _cache["nc"]

    nc = bacc.Bacc("TRN2", target_bir_lowering=False, debug=False)

    qT = nc.dram_tensor("qT", [D, L], F32, kind="ExternalInput").ap()
    kT = nc.dram_tensor("kT", [D, L], F32, kind="ExternalInput").ap()
    vT = nc.dram_tensor("vT", [D, L], F32, kind="ExternalInput").ap()
    wqT = nc.dram_tensor("wqT", [D, DH], F32, kind="ExternalInput").ap()
    wkT = nc.dram_tensor("wkT", [D, DH], F32, kind="ExternalInput").ap()
    wvT = nc.dram_tensor("wvT", [D, DH], F32, kind="ExternalInput").ap()
    woT = nc.dram_tensor("woT", [DH, D], F32, kind="ExternalInput").ap()
    bqc = nc.dram_tensor("bqc", [128, 4], F32, kind="ExternalInput").ap()
    bkc = nc.dram_tensor("bkc", [128, 4], F32, kind="ExternalInput").ap()
    bvc = nc.dram_tensor("bvc", [1, DH], F32, kind="ExternalInput").ap()
    cosP = nc.dram_tensor("cosP", [128, L], F32, kind="ExternalInput").ap()
    sinP = nc.dram_tensor("sinP", [128, L], F32, kind="ExternalInput").ap()
    maskc = nc.dram_tensor("maskc", [128, 128], F32, kind="ExternalInput").ap()
    outT = nc.dram_tensor("outT", [D, L], F32, kind="ExternalOutput").ap()

    with tile.TileContext(nc) as tc, ExitStack() as ctx:
        const = ctx.enter_context(tc.tile_pool(name="const", bufs=1))

        mask_t = const.tile([128, 128], F32, tag="mask")
        nc.sync.dma_start(mask_t[:], maskc[:])
        bq_t = const.tile([128, 4], F32, tag="bq")
        nc.sync.dma_start(bq_t[:], bqc[:])
        bk_t = const.tile([128, 4], F32, tag="bk")
        nc.sync.dma_start(bk_t[:], bkc[:])
        bv_sb = const.tile([1, DH], F32, tag="bv")
        nc.sync.dma_start(bv_sb[:], bvc[:])
        bv_b = const.tile([128, DH], F32, tag="bvb")
        nc.gpsimd.partition_broadcast(bv_b[:], bv_sb[:])

        with ExitStack() as bctx:
            pqk = bctx.enter_context(tc.tile_pool(name="pqk", bufs=1))
            pva = bctx.enter_context(tc.tile_pool(name="pva", bufs=1))
            qt_m = [pqk.tile([128, L], F32, tag=f"qt{m}", name=f"qt{m}") for m in range(4)]
            kt_m = [pqk.tile([128, L], F32, tag=f"kt{m}", name=f"kt{m}") for m in range(4)]
            va = [pva.tile([128, HPC * 65], F32, tag=f"va{t}", name=f"va{t}") for t in range(KT)]
            for t in range(KT):
                ones_view = va[t].rearrange("p (h x) -> p h x", x=65)[:, :, 64:65]
                nc.gpsimd.memset(ones_view, 1.0)

            # ---------------- Phase A1: Q/K projections + RoPE ------------
            with ExitStack() as actx:
                pin = actx.enter_context(tc.tile_pool(name="pin", bufs=2))
                pw = actx.enter_context(tc.tile_pool(name="pw", bufs=1))
                ptrig = actx.enter_context(tc.tile_pool(name="ptrig", bufs=1))
                psw = actx.enter_context(tc.tile_pool(name="psw", bufs=2))
                psA = actx.enter_context(
                    tc.tile_pool(name="psA", bufs=2, space="PSUM"))

                cos_t = ptrig.tile([128, L], F32, tag="cos")
                nc.sync.dma_start(cos_t[:], cosP[:])
                sin_t = ptrig.tile([128, L], F32, tag="sin")
                nc.sync.dma_start(sin_t[:], sinP[:])

                for xT, wT, bias_t, dst in (
                    (kT, wkT, bk_t, kt_m),
                    (qT, wqT, bq_t, qt_m),
                ):
                    # weights: 8 k-tiles [128, 512], live for this tensor
                    w_sb = []
                    for kk in range(8):
                        w = pw.tile([128, DH], F32, tag=f"w{kk}")
                        nc.sync.dma_start(
                            _r(w), _r(wT[kk * 128:(kk + 1) * 128, :]))
                        w_sb.append(w)
                    for nj in range(4):
                        xp = []
                        for kk in range(8):
                            x = pin.tile([128, 512], F32, tag=f"x{kk}")
                            nc.sync.dma_start(
                                _r(x),
                                _r(xT[kk * 128:(kk + 1) * 128,
                                      nj * 512:(nj + 1) * 512]))
                            xp.append(x)
                        for mi in range(4):
                            ps = psA.tile([128, 512], F32, tag=f"pp{mi}")
                            for kk in range(8):
                                nc.tensor.matmul(
                                    ps[:],
                                    _r(w_sb[kk][:, mi * 128:(mi + 1) * 128]),
                                    _r(xp[kk][:]),
                                    start=(kk == 0), stop=(kk == 7),
                                )
                            # PSUM -> SBUF with per-partition bias
                            nc.vector.tensor_scalar_add(
                                _r(dst[mi][:, nj * 512:(nj + 1) * 512]),
                                ps[:], bias_t[:, mi:mi + 1])
                    # RoPE per m-tile (after all 4 chunks written)
                    for mi in range(4):
                        sw = psw.tile([128, L], F32, tag="sw")
                        for blk in range(4):
                            srcb = blk ^ 1
                            nc.sync.dma_start(
                                sw[blk * 32:(blk + 1) * 32, :],
                                dst[mi][srcb * 32:(srcb + 1) * 32, :])
                        nc.vector.tensor_mul(sw[:], sw[:], sin_t[:])
                        nc.vector.tensor_mul(_r(dst[mi][:]), dst[mi][:],
                                             cos_t[:])
                        nc.vector.tensor_add(_r(dst[mi][:]), dst[mi][:],
                                             sw[:])

            # ---------------- Phase A2: V projection (natural layout) -----
            with ExitStack() as actx:
                pinv = actx.enter_context(tc.tile_pool(name="pinv", bufs=2))
                pwv = actx.enter_context(tc.tile_pool(name="pwv", bufs=1))
                psV = actx.enter_context(
                    tc.tile_pool(name="psV", bufs=2, space="PSUM"))

                wv_sb = []
                for kk in range(8):
                    w = pwv.tile([128, DH], F32, tag=f"wv{kk}")
                    nc.sync.dma_start(_r(w), _r(wvT[kk * 128:(kk + 1) * 128, :]))
                    wv_sb.append(w)
                for ltg in range(4):           # groups of 4 l-tiles
                    vp = []
                    for kk in range(8):
                        x = pinv.tile([128, 512], F32, tag=f"vx{kk}")
                        nc.sync.dma_start(
                            _r(x),
                            _r(vT[kk * 128:(kk + 1) * 128,
                                  ltg * 512:(ltg + 1) * 512]))
                        vp.append(x)
                    for li in range(4):
                        lt = ltg * 4 + li
                        ps = psV.tile([128, DH], F32, tag=f"pv{li}")
                        for kk in range(8):
                            nc.tensor.matmul(
                                ps[:],
                                _r(vp[kk][:, li * 128:(li + 1) * 128]),
                                _r(wv_sb[kk][:]),
                                start=(kk == 0), stop=(kk == 7),
                            )
                        out_view = va[lt].rearrange(
                            "p (h x) -> p h x", x=65)[:, :, 0:64]
                        nc.vector.tensor_add(_r(out_view), ps[:], bv_b[:])

            # ot pool opens after phase-A pools close (stack allocator)
            pot = bctx.enter_context(tc.tile_pool(name="pot", bufs=1))
            ot_m = [pot.tile([128, L], F32, tag=f"ot{m}", name=f"ot{m}")
                    for m in range(4)]

            # ---------------- Phase B: attention per head -----------------
            with ExitStack() as bctx2:
                psS = bctx2.enter_context(
                    tc.tile_pool(name="psS", bufs=1, space="PSUM"))
                psO = bctx2.enter_context(
                    tc.tile_pool(name="psO", bufs=1, space="PSUM"))
                ppp = bctx2.enter_context(tc.tile_pool(name="ppp", bufs=2))
                pnm = bctx2.enter_context(tc.tile_pool(name="pnm", bufs=1))

                for h in range(HPC):
                    mi, pb = h // 2, (h % 2) * 64
                    o_ps = psO.tile([128, L], F32, tag="O")
                    for kt_i in range(KT):
                        qoff = kt_i * 128
                        w = L - qoff
                        s_ps = psS.tile([128, L], F32, tag="S")
                        nch = (w + 511) // 512
                        for c in range(nch):
                            cw = min(512, w - c * 512)
                            nc.tensor.matmul(
                                s_ps[:, c * 512:c * 512 + cw],
                                _r(kt_m[mi][pb:pb + 64,
                                            qoff:qoff + 128]),
                                _r(qt_m[mi][pb:pb + 64,
                                            qoff + c * 512:
                                            qoff + c * 512 + cw]),
                                start=True, stop=True,
                            )
                        pt = ppp.tile([128, L], F32, tag="P")
                        nc.scalar.activation(_r(pt[:, :w]), s_ps[:, :w],
                                             AF.Exp, scale=0.125)
                        nc.vector.tensor_mul(_r(pt[:, 0:128]), pt[:, 0:128],
                                             mask_t[:])
                        for qc in range(kt_i // 4, 4):
                            lo = max(qc * 512, qoff)
                            hi = qc * 512 + 512
                            nc.tensor.matmul(
                                o_ps[0:65, lo:hi],
                                _r(va[kt_i][:, h * 65:h * 65 + 65]),
                                _r(pt[:, lo - qoff:hi - qoff]),
                                start=(kt_i == 0), stop=(kt_i == 4 * qc + 3),
                                skip_group_check=True,
                            )
                    # normalize: o / colsum, write into ot_m slot
                    rc = pnm.tile([1, L], F32, tag="rc")
                    nc.vector.tensor_copy(rc[:], o_ps[64:65, :])
                    nc.vector.reciprocal(rc[:], rc[:])
                    rcb = pnm.tile([64, L], F32, tag="rcb")
                    nc.gpsimd.partition_broadcast(rcb[:], rc[:], channels=64)
                    nc.vector.tensor_mul(_r(ot_m[mi][pb:pb + 64, :]),
                                         o_ps[0:64, :], rcb[:])

            # ------------- Phase C: output projection ---------------------
            with ExitStack() as cctx:
                pwo = cctx.enter_context(tc.tile_pool(name="pwo", bufs=1))
                pout = cctx.enter_context(tc.tile_pool(name="pout", bufs=2))
                psC = cctx.enter_context(
                    tc.tile_pool(name="psC", bufs=2, space="PSUM"))

                wo_sb = []
                for kti in range(4):
                    w = pwo.tile([128, D], F32, tag=f"wo{kti}")
                    nc.sync.dma_start(
                        _r(w), _r(woT[kti * 128:(kti + 1) * 128, :]))
                    wo_sb.append(w)
                for mo in range(8):
                    ps = psC.tile([128, L], F32, tag="C")
                    for nj in range(4):
                        for kti in range(4):
                            nc.tensor.matmul(
                                ps[:, nj * 512:(nj + 1) * 512],
                                _r(wo_sb[kti][:, mo * 128:(mo + 1) * 128]),
                                _r(ot_m[kti][:, nj * 512:(nj + 1) * 512]),
                                start=(kti == 0), stop=(kti == 3),
                            )
                    osb = pout.tile([128, L], F32, tag="osb")
                    nc.scalar.copy(osb[:], ps[:])
                    nc.sync.dma_start(outT[mo * 128:(mo + 1) * 128, :],
                                      osb[:])

    if compile:
        nc.compile()
        _cache["nc"] = nc
    return nc


def _prep(q, k, v, Wq, bq, Wk, bk, Wv, bv, Wo, bo):
    """Build the 8 per-core input maps (host-side shard + layout prep)."""
    # de-interleave permutation within each head: evens then odds
    perm = np.concatenate([np.arange(0, HD, 2), np.arange(1, HD, 2)])

    # RoPE tables in de-interleaved layout, tiled x2 over partitions
    inv_freq = 1.0 / (ROPE_BASE ** (np.arange(0, HD // 2, dtype=np.float64)
                                    * 2.0 / HD))
    t = np.arange(L, dtype=np.float64)
    freqs = inv_freq[:, None] * t[None, :]            # [32, L]
    cos64 = np.cos(np.concatenate([freqs, freqs], axis=0))   # [64, L]
    sin64 = np.sin(np.concatenate([freqs, freqs], axis=0))
    sin64[:32] *= -1.0
    cosP = np.tile(cos64, (2, 1)).astype(np.float32)  # [128, L]
    sinP = np.tile(sin64, (2, 1)).astype(np.float32)

    # causal mask in S^T space: keep k <= q
    kk, qq = np.meshgrid(np.arange(128), np.arange(128), indexing="ij")
    mask = (kk <= qq).astype(np.float32)

    in_maps = []
    for c in range(NCORES):
        b_i, hg = c // 2, c % 2
        rows = hg * DH + (np.arange(DH).reshape(HPC, HD)[:, perm]).reshape(-1)
        wq_s = Wq[rows, :]                      # [512, 1024] permuted rows
        wk_s = Wk[rows, :]
        bq_s = bq[rows]
        bk_s = bk[rows]
        wv_s = Wv[hg * DH:(hg + 1) * DH, :]     # natural
        bv_s = bv[hg * DH:(hg + 1) * DH]
        wo_s = Wo[:, hg * DH:(hg + 1) * DH]     # [1024, 512]

        in_maps.append({
            "qT": np.ascontiguousarray(q[b_i].T),
            "kT": np.ascontiguousarray(k[b_i].T),
            "vT": np.ascontiguousarray(v[b_i].T),
            "wqT": np.ascontiguousarray(wq_s.T),
            "wkT": np.ascontiguousarray(wk_s.T),
            "wvT": np.ascontiguousarray(wv_s.T),
            "woT": np.ascontiguousarray(wo_s.T),
            "bqc": np.ascontiguousarray(bq_s.reshape(4, 128).T),
            "bkc": np.ascontiguousarray(bk_s.reshape(4, 128).T),
            "bvc": np.ascontiguousarray(bv_s.reshape(1, DH)),
            "cosP": cosP,
            "sinP": sinP,
            "maskc": mask,
        })
    return in_maps


def _assemble(results, bo):
    out = np.empty((B, L, D), dtype=np.float32)
    for b_i in range(B):
        acc = results[2 * b_i]["outT"] + results[2 * b_i + 1]["outT"]
        out[b_i] = acc.T + bo[None, :]
    return out


def kernel(q, k, v, Wq, bq, Wk, bk, Wv, bv, Wo, bo):
    q = np.asarray(q, dtype=np.float32)
    k = np.asarray(k, dtype=np.float32)
    v = np.asarray(v, dtype=np.float32)
    Wq = np.asarray(Wq, dtype=np.float32)
    Wk = np.asarray(Wk, dtype=np.float32)
    Wv = np.asarray(Wv, dtype=np.float32)
    Wo = np.asarray(Wo, dtype=np.float32)
    bq = np.asarray(bq, dtype=np.float32)
    bk = np.asarray(bk, dtype=np.float32)
    bv = np.asarray(bv, dtype=np.float32)
    bo = np.asarray(bo, dtype=np.float32)

    nc = _build()
    in_maps = _prep(q, k, v, Wq, bq, Wk, bk, Wv, bv, Wo, bo)
    res = run_bass_kernel_spmd(nc, in_maps, core_ids=list(range(NCORES)))
    return _assemble(res.results, bo)
